# revision 15
# baseline (speedup 1.0000x reference)
"""Conv2d(128->256, 3x3, pad 1) + bias on 16x128x56x56, SPMD over 8 TRN2 cores.

Data-parallel over batch: each core convolves 2 images. Per core the conv is
an implicit GEMM: contraction over CIN=128 (the SBUF partition dim), with the
9 kernel taps accumulated into PSUM via start/stop matmul groups.

Zero-padding: x tiles are padded in W only ([128, 56, 58], two pad columns
memset once); H padding is handled with NO pad rows - the kh=0 / kh=2 taps
of the first/last block restrict their output to the rows whose input lies
inside the image (the dropped rows are exactly where the reference
multiplies by zero padding). Row-restricted PSUM regions stay contiguous,
which the fp32r ISA dst-pattern check requires (column-restricted regions
are strided and get rejected). The center tap (kh=1) covers the full block
and runs first with start=True so every PSUM element is initialized. This
kills the staging copy + GPSIMD repack of the earlier design - images DMA
straight into the padded interior (8x224B segments per packet run at full
DMA rate, same as the output drains). Free size stays >= 392 (full fp32r
rate needs >= 256).

Startup schedule (driven by NTFF profile analysis of the old kernel):
- Engine preambles end ~7us; nothing can issue before that.
- Descriptor generation occupies the issuing engine ~6ns/descriptor and
  descriptors = partitions x segments, so weights are laid out on the host
  as [ci, t, k, co] making each t-half ONE contiguous 128-descriptor DMA
  (0.7us gen) instead of 9 segments/partition (3.8us gen).
- ACT queue order = consumption order: w(t=0), img0 rows 0-8, rows 9-16,
  bias, rows 17-24, w(t=1), remaining img0 chunks.
- Image 1 is emitted AFTER the first drain on the SYNC queue: its 1.6MB
  bulk transfer otherwise saturates the DMA fabric exactly when the
  latency-critical first chunks + weights are in flight (measured +3us).
- Warmup matmuls use iota-filled (nonzero, varying) data: the HAM power
  monitor ignores all-zero matmuls (no bit toggling -> no power draw), which
  left the PE at half clock for the first 13us and made the old 40-matmul
  zero warmup useless. HAM ramps ~4us after sustained real activity begins.
"""

import numpy as np

B, CIN, COUT, H, W = 16, 128, 256, 56, 56
KH, KW = 3, 3
N_CORES = 8
IMGS_PER_CORE = B // N_CORES  # 2
CO_TILES = COUT // 128  # 2
RB = 8  # output rows per PSUM block
NB = H // RB  # 7 blocks
K_ORDER = [4, 0, 1, 2, 3, 5, 6, 7, 8]  # center tap first: full PSUM coverage
N_WARM = 5

# image-0 row chunks: 9 rows first (block 0 needs rows 0-8), then 8-row steps
CHUNKS0 = [(0, 9), (9, 17), (17, 25), (25, 33), (33, 41), (41, 49), (49, 56)]

_COMPILED = {}


WP = W + 2  # x tiles are W-padded only


def _tap(rb, k):
    """Output row range [r0,r1) for tap k of block rb; rows whose input
    would fall in the (nonexistent) H padding are excluded. Row-restricted
    PSUM regions stay contiguous, as the fp32r dst-pattern check needs."""
    kh, kw = divmod(k, KW)
    h0 = rb * RB
    r0 = h0 + 1 if (kh == 0 and rb == 0) else h0
    r1 = H - 1 if (kh == 2 and rb == NB - 1) else h0 + RB
    return kh, kw, h0, r0, r1


def _build(mm_dtype_name: str):
    import concourse.bacc as bacc
    import concourse.mybir as mybir
    import concourse.tile as tile

    mm_dt = getattr(mybir.dt, mm_dtype_name)
    f32 = mybir.dt.float32
    four_byte = mybir.dt.size(mm_dt) == 4
    in_dt = f32 if four_byte else mm_dt

    def mm_view(ap):
        return ap.bitcast(mm_dt) if four_byte else ap

    nc = bacc.Bacc("TRN2", target_bir_lowering=False, debug=False,
                   num_devices=N_CORES)
    x_dram = nc.dram_tensor("x", [IMGS_PER_CORE, CIN, H, W], in_dt,
                            kind="ExternalInput").ap()
    w_dram = nc.dram_tensor("w", [CIN, CO_TILES * KH * KW * 128], in_dt,
                            kind="ExternalInput").ap()
    b_dram = nc.dram_tensor("b", [128, CO_TILES], f32,
                            kind="ExternalInput").ap()
    out_dram = nc.dram_tensor("out", [IMGS_PER_CORE, COUT, H, W], f32,
                              kind="ExternalOutput").ap()

    # host layout [ci, t, i, co] with i = position in K_ORDER
    w_dram_v = mm_view(w_dram).rearrange(
        "c (t k o) -> c t k o", t=CO_TILES, k=KH * KW)

    with tile.TileContext(nc) as tc:
        with (
            tc.tile_pool(name="xp", bufs=1) as x_pool,
            tc.tile_pool(name="wp", bufs=1) as w_pool,
            tc.tile_pool(name="op", bufs=8) as out_pool,
            tc.tile_pool(name="ps", bufs=7, space="PSUM") as psum_pool,
        ):
            # PE warmup on nonzero varying data (iota): ramps the HAM clock
            # while the first image chunk + weights are in flight.
            junk_u = w_pool.tile([128, 448], mybir.dt.uint32, tag="junku")
            junk = w_pool.tile([128, 448], mm_dt, tag="junk")
            nc.gpsimd.iota(junk_u[:], pattern=[[1, 448]], channel_multiplier=3)
            nc.gpsimd.tensor_copy(junk[:], junk_u[:])  # int -> real normals
            junk_mm = junk[:]
            wpsum = psum_pool.tile([128, 448], f32, tag="warm", bufs=1)
            for _ in range(N_WARM):
                nc.tensor.matmul(wpsum[:], junk_mm[:, :128],
                                 junk_mm[:, :448], start=True, stop=True)

            w_sb = w_pool.tile([CIN, CO_TILES, KH * KW, 128], mm_dt)
            b_sb = w_pool.tile([128, CO_TILES], f32, tag="bias")
            x0 = x_pool.tile([CIN, H, WP], mm_dt, tag="x0")
            x1 = x_pool.tile([CIN, H, WP], mm_dt, tag="x1")
            xs = [x0, x1]
            zero_dt = mybir.dt.uint32 if four_byte else mybir.dt.uint16
            for xi in xs:
                nc.vector.memset(xi[:, :, 0].bitcast(zero_dt), 0)
                nc.vector.memset(xi[:, :, WP - 1].bitcast(zero_dt), 0)

            def chunk_dma(img, ci):
                r0, r1 = CHUNKS0[ci]
                nc.scalar.dma_start(xs[img][:, r0:r1, 1:WP - 1],
                                    mm_view(x_dram[img, :, r0:r1, :]))

            # ACT queue, in consumption order (each item one dense DMA)
            nc.scalar.dma_start(w_sb[:, 0, :, :], w_dram_v[:, 0, :, :])
            chunk_dma(0, 0)
            chunk_dma(0, 1)
            nc.scalar.dma_start(b_sb[:], b_dram[:])
            chunk_dma(0, 2)
            nc.scalar.dma_start(w_sb[:, 1, :, :], w_dram_v[:, 1, :, :])
            for ci in range(3, len(CHUNKS0)):
                chunk_dma(0, ci)

            def group(img, rb, t):
                psum = psum_pool.tile([128, RB, W], f32)
                x = xs[img]
                for i, k in enumerate(K_ORDER):
                    kh, kw, h0, r0, r1 = _tap(rb, k)
                    nc.tensor.matmul(
                        psum[:, r0 - h0:r1 - h0, :],
                        w_sb[:, t, i, :],
                        x[:, r0 - 1 + kh:r1 - 1 + kh, kw:kw + W],
                        start=(i == 0),
                        stop=(i == len(K_ORDER) - 1),
                    )
                out_sb = out_pool.tile([128, RB, W], f32)
                nc.vector.tensor_scalar_add(out_sb[:], psum[:],
                                            b_sb[:, t:t + 1])
                nc.sync.dma_start(
                    out_dram[img, t * 128:(t + 1) * 128, h0:h0 + RB, :],
                    out_sb[:])

            group(0, 0, 0)
            # image 1 rides SYNC behind the first drain, keeping its 1.6MB
            # off the DMA fabric during the latency-critical startup window
            nc.sync.dma_start(x1[:, :, 1:WP - 1], mm_view(x_dram[1]))
            for rb in range(1, NB):
                group(0, rb, 0)
            for rb in range(NB):
                group(0, rb, 1)
            for t in range(CO_TILES):
                for rb in range(NB):
                    group(1, rb, t)
    nc.compile()
    return nc


def _get_nc(mm_dtype_name: str):
    if mm_dtype_name not in _COMPILED:
        _COMPILED[mm_dtype_name] = _build(mm_dtype_name)
    return _COMPILED[mm_dtype_name]


def prep_inputs(x, weight, bias, mm_dtype_name="float32r"):
    """Shard/transform full inputs into per-core in_maps."""
    if mm_dtype_name == "bfloat16":
        import ml_dtypes
        in_np = ml_dtypes.bfloat16
    else:
        in_np = np.float32
    x = np.ascontiguousarray(np.asarray(x, dtype=np.float32).astype(in_np))
    # [co, ci, kh, kw] -> [ci, t, K_ORDER-permuted tap, co'] (contiguous
    # per-t half: one descriptor per partition per DMA)
    w4 = (np.asarray(weight, dtype=np.float32)
          .reshape(CO_TILES, 128, CIN, KH * KW))[:, :, :, K_ORDER]
    w_prep = np.ascontiguousarray(
        w4.transpose(2, 0, 3, 1)
        .reshape(CIN, CO_TILES * KH * KW * 128).astype(in_np))
    b_prep = np.ascontiguousarray(
        np.asarray(bias, dtype=np.float32)
        .reshape(CO_TILES, 128).transpose(1, 0))
    return [
        {"x": x[c * IMGS_PER_CORE:(c + 1) * IMGS_PER_CORE],
         "w": w_prep, "b": b_prep}
        for c in range(N_CORES)
    ]


def run(x, weight, bias, mm_dtype_name="float32r", trace=False):
    from concourse.bass_utils import run_bass_kernel_spmd
    nc = _get_nc(mm_dtype_name)
    in_maps = prep_inputs(x, weight, bias, mm_dtype_name)
    res = run_bass_kernel_spmd(nc, in_maps, list(range(N_CORES)), trace=trace)
    out = np.concatenate([res.results[c]["out"] for c in range(N_CORES)],
                         axis=0)
    return out, res


def kernel(x, weight, bias):
    out, _ = run(np.asarray(x), np.asarray(weight), np.asarray(bias))
    return out


# revision 18
# speedup vs baseline: 1.0232x; 1.0232x over previous
"""Conv2d(128->256, 3x3, pad 1) + bias on 16x128x56x56, SPMD over 8 TRN2 cores.

Data-parallel over batch: each core convolves 2 images. Per core the conv is
an implicit GEMM: contraction over CIN=128 (the SBUF partition dim), with the
9 kernel taps accumulated into PSUM via start/stop matmul groups.

Zero-padding: x is padded in W on the HOST ([.., 56, 58]) so every image
DMA is fully dense on both sides (strided SBUF destinations fragment into
224-byte per-segment packets that run at ~1/3 fabric bandwidth and, worse,
seem to anchor the HAM-chosen PE clock at ~2.0GHz instead of 2.4). H
padding needs NO pad rows: the kh=0 / kh=2 taps of the first/last block
restrict their output to the rows whose input lies inside the image (the
dropped rows are exactly where the reference multiplies by zero padding).
Row-restricted PSUM regions stay contiguous, which the fp32r ISA
dst-pattern check requires (column-restricted regions are strided and get
rejected). The center tap (kh=1) covers the full block and runs first with
start=True so every PSUM element is initialized. No staging copies, no
repacks, no memsets. Free size stays >= 392 (full fp32r rate needs >= 256).

Startup schedule (driven by NTFF profile analysis of the old kernel):
- Engine preambles end ~7us; nothing can issue before that.
- Descriptor generation occupies the issuing engine ~6ns/descriptor and
  descriptors = partitions x segments, so weights are laid out on the host
  as [ci, t, k, co] making each t-half ONE contiguous 128-descriptor DMA
  (0.7us gen) instead of 9 segments/partition (3.8us gen).
- ACT queue order = consumption order: w(t=0), img0 rows 0-8, rows 9-16,
  bias, rows 17-24, w(t=1), remaining img0 chunks.
- Image 1 is emitted AFTER the first drain on the SYNC queue: its 1.6MB
  bulk transfer otherwise saturates the DMA fabric exactly when the
  latency-critical first chunks + weights are in flight (measured +3us).
- Warmup matmuls use iota-filled (nonzero, varying) data: the HAM power
  monitor ignores all-zero matmuls (no bit toggling -> no power draw), which
  left the PE at half clock for the first 13us and made the old 40-matmul
  zero warmup useless. HAM ramps ~4us after sustained real activity begins.
"""

import numpy as np

B, CIN, COUT, H, W = 16, 128, 256, 56, 56
KH, KW = 3, 3
N_CORES = 8
IMGS_PER_CORE = B // N_CORES  # 2
CO_TILES = COUT // 128  # 2
RB = 8  # output rows per PSUM block
NB = H // RB  # 7 blocks
K_ORDER = [4, 0, 1, 2, 3, 5, 6, 7, 8]  # center tap first: full PSUM coverage
N_WARM = 5

# image-0 row chunks: 9 rows first (block 0 needs rows 0-8), then 8-row steps
CHUNKS0 = [(0, 9), (9, 17), (17, 25), (25, 33), (33, 41), (41, 49), (49, 56)]

_COMPILED = {}


WP = W + 2  # x tiles are W-padded only


def _tap(rb, k):
    """Output row range [r0,r1) for tap k of block rb; rows whose input
    would fall in the (nonexistent) H padding are excluded. Row-restricted
    PSUM regions stay contiguous, as the fp32r dst-pattern check needs."""
    kh, kw = divmod(k, KW)
    h0 = rb * RB
    r0 = h0 + 1 if (kh == 0 and rb == 0) else h0
    r1 = H - 1 if (kh == 2 and rb == NB - 1) else h0 + RB
    return kh, kw, h0, r0, r1


def _build(mm_dtype_name: str):
    import concourse.bacc as bacc
    import concourse.mybir as mybir
    import concourse.tile as tile

    mm_dt = getattr(mybir.dt, mm_dtype_name)
    f32 = mybir.dt.float32
    four_byte = mybir.dt.size(mm_dt) == 4
    in_dt = f32 if four_byte else mm_dt

    def mm_view(ap):
        return ap.bitcast(mm_dt) if four_byte else ap

    nc = bacc.Bacc("TRN2", target_bir_lowering=False, debug=False,
                   num_devices=N_CORES)
    x_dram = nc.dram_tensor("x", [IMGS_PER_CORE, CIN, H, WP], in_dt,
                            kind="ExternalInput").ap()
    w_dram = nc.dram_tensor("w", [CIN, CO_TILES * KH * KW * 128], in_dt,
                            kind="ExternalInput").ap()
    b_dram = nc.dram_tensor("b", [128, CO_TILES], f32,
                            kind="ExternalInput").ap()
    out_dram = nc.dram_tensor("out", [IMGS_PER_CORE, COUT, H, W], f32,
                              kind="ExternalOutput").ap()

    # host layout [ci, t, i, co] with i = position in K_ORDER
    w_dram_v = mm_view(w_dram).rearrange(
        "c (t k o) -> c t k o", t=CO_TILES, k=KH * KW)

    with tile.TileContext(nc) as tc:
        with (
            tc.tile_pool(name="xp", bufs=1) as x_pool,
            tc.tile_pool(name="wp", bufs=1) as w_pool,
            tc.tile_pool(name="op", bufs=8) as out_pool,
            tc.tile_pool(name="ps", bufs=7, space="PSUM") as psum_pool,
        ):
            # PE warmup on nonzero varying data (iota): ramps the HAM clock
            # while the first image chunk + weights are in flight.
            junk_u = w_pool.tile([128, 448], mybir.dt.uint32, tag="junku")
            junk = w_pool.tile([128, 448], mm_dt, tag="junk")
            nc.gpsimd.iota(junk_u[:], pattern=[[1, 448]], channel_multiplier=3)
            nc.gpsimd.tensor_copy(junk[:], junk_u[:])  # int -> real normals
            junk_mm = junk[:]
            wpsum = psum_pool.tile([128, 448], f32, tag="warm", bufs=1)
            for _ in range(N_WARM):
                nc.tensor.matmul(wpsum[:], junk_mm[:, :128],
                                 junk_mm[:, :448], start=True, stop=True)

            w_sb = w_pool.tile([CIN, CO_TILES, KH * KW, 128], mm_dt)
            b_sb = w_pool.tile([128, CO_TILES], f32, tag="bias")
            x0 = x_pool.tile([CIN, H, WP], mm_dt, tag="x0")
            x1 = x_pool.tile([CIN, H, WP], mm_dt, tag="x1")
            xs = [x0, x1]
            zero_dt = mybir.dt.uint32 if four_byte else mybir.dt.uint16

            def chunk_dma(img, ci):
                r0, r1 = CHUNKS0[ci]
                nc.scalar.dma_start(xs[img][:, r0:r1, :],
                                    mm_view(x_dram[img, :, r0:r1, :]))

            # ACT queue, in consumption order (each item one dense DMA)
            nc.scalar.dma_start(w_sb[:, 0, :, :], w_dram_v[:, 0, :, :])
            chunk_dma(0, 0)
            chunk_dma(0, 1)
            chunk_dma(0, 2)
            nc.scalar.dma_start(w_sb[:, 1, :, :], w_dram_v[:, 1, :, :])
            for ci in range(3, len(CHUNKS0)):
                chunk_dma(0, ci)
            # bias rides the otherwise-idle GPSIMD SWDGE queue
            nc.gpsimd.dma_start(b_sb[:], b_dram[:])

            def group(img, rb, t):
                psum = psum_pool.tile([128, RB, W], f32)
                x = xs[img]
                for i, k in enumerate(K_ORDER):
                    kh, kw, h0, r0, r1 = _tap(rb, k)
                    nc.tensor.matmul(
                        psum[:, r0 - h0:r1 - h0, :],
                        w_sb[:, t, i, :],
                        x[:, r0 - 1 + kh:r1 - 1 + kh, kw:kw + W],
                        start=(i == 0),
                        stop=(i == len(K_ORDER) - 1),
                    )
                out_sb = out_pool.tile([128, RB, W], f32)
                nc.vector.tensor_scalar_add(out_sb[:], psum[:],
                                            b_sb[:, t:t + 1])
                nc.sync.dma_start(
                    out_dram[img, t * 128:(t + 1) * 128, h0:h0 + RB, :],
                    out_sb[:])

            group(0, 0, 0)
            # Keep image 1 off the DMA fabric during the latency-critical
            # startup window: the scheduler hoists ready DMAs above waiting
            # instructions, so delay it with a WAW dependency - a DVE memset
            # overlapping its first columns that sits behind g1's drain in
            # the DVE stream (the DMA then overwrites col 1 with real data).
            nc.vector.memset(x1[:, :, 0:2].bitcast(zero_dt), 0)
            nc.sync.dma_start(x1[:], mm_view(x_dram[1]))
            for rb in range(1, NB):
                group(0, rb, 0)
            for rb in range(NB):
                group(0, rb, 1)
            for t in range(CO_TILES):
                for rb in range(NB):
                    group(1, rb, t)
    nc.compile()
    return nc


def _get_nc(mm_dtype_name: str):
    if mm_dtype_name not in _COMPILED:
        _COMPILED[mm_dtype_name] = _build(mm_dtype_name)
    return _COMPILED[mm_dtype_name]


def prep_inputs(x, weight, bias, mm_dtype_name="float32r"):
    """Shard/transform full inputs into per-core in_maps."""
    if mm_dtype_name == "bfloat16":
        import ml_dtypes
        in_np = ml_dtypes.bfloat16
    else:
        in_np = np.float32
    # pad W on the host: pad columns arrive pre-zeroed from DRAM, and every
    # image DMA lands dense in SBUF (one descriptor/partition, no 224B
    # segment packets at 1/3 fabric bandwidth)
    x = np.pad(np.asarray(x, dtype=np.float32), ((0, 0), (0, 0), (0, 0), (1, 1)))
    x = np.ascontiguousarray(x.astype(in_np))
    # [co, ci, kh, kw] -> [ci, t, K_ORDER-permuted tap, co'] (contiguous
    # per-t half: one descriptor per partition per DMA)
    w4 = (np.asarray(weight, dtype=np.float32)
          .reshape(CO_TILES, 128, CIN, KH * KW))[:, :, :, K_ORDER]
    w_prep = np.ascontiguousarray(
        w4.transpose(2, 0, 3, 1)
        .reshape(CIN, CO_TILES * KH * KW * 128).astype(in_np))
    b_prep = np.ascontiguousarray(
        np.asarray(bias, dtype=np.float32)
        .reshape(CO_TILES, 128).transpose(1, 0))
    return [
        {"x": x[c * IMGS_PER_CORE:(c + 1) * IMGS_PER_CORE],
         "w": w_prep, "b": b_prep}
        for c in range(N_CORES)
    ]


def run(x, weight, bias, mm_dtype_name="float32r", trace=False):
    from concourse.bass_utils import run_bass_kernel_spmd
    nc = _get_nc(mm_dtype_name)
    in_maps = prep_inputs(x, weight, bias, mm_dtype_name)
    res = run_bass_kernel_spmd(nc, in_maps, list(range(N_CORES)), trace=trace)
    out = np.concatenate([res.results[c]["out"] for c in range(N_CORES)],
                         axis=0)
    return out, res


def kernel(x, weight, bias):
    out, _ = run(np.asarray(x), np.asarray(weight), np.asarray(bias))
    return out
